# revision 12
# baseline (speedup 1.0000x reference)
"""CAM (channel attention) kernel for Trainium2, data-parallel over batch.

out[b] = gamma * (a[b] @ softmax(a[b]^T a[b])) + x[b],  a[b] = x[b].reshape(HW, C)

Per core (one batch element):
  Phase A: stream a into a resident SBUF buffer in [128, 512] double-tiles
           and accumulate aTa = a^T a in PSUM (f32r matmuls, K=16384 over
           128 accumulation steps). DMA-bound.
  Softmax: row-softmax of aTa folded into M = gamma * attn + I, so
           out = a @ M (residual + gamma fused into the small matrix).
  Phase B: per double-tile, PE-transpose a-chunks (f32r transpose-mode,
           value-preserving) into a packed PSUM bank, evacuate to a small
           aT ring (DVE+ACT halves), then out rows = aT_chunk.T @ M
           (f32r matmuls, K=256), evacuate PSUM -> SBUF -> DRAM.
           The transposes only depend on a, so the PE works through the
           softmax barrier without going cold.
Dummy bf16 matmuls warm the PE clock gate (HAM) at kernel start.
"""

import sys

import numpy as np

for _p in ("/opt/trn_rl_repo",):
    if _p not in sys.path:
        sys.path.insert(0, _p)

import concourse.bass as bass
import concourse.tile as tile
from concourse import bacc, mybir
from concourse.bass_utils import run_bass_kernel_spmd

B, H, W, C = 8, 128, 128, 256
HW = H * W
P = 128
NT = HW // P          # 128 row-tiles of a
ND = NT // 2          # 64 double-tiles
N_CORES = 8

f32 = mybir.dt.float32
f32r = mybir.dt.float32r
bf16 = mybir.dt.bfloat16
ts = bass.ts


def _cam_body(tc, y_out, x_in, g_in):
    nc = tc.nc
    import contextlib

    with contextlib.ExitStack() as ctx:
        const = ctx.enter_context(tc.tile_pool(name="const", bufs=1))
        abig = ctx.enter_context(tc.tile_pool(name="abig", bufs=1))
        tring = ctx.enter_context(tc.tile_pool(name="tring", bufs=4))
        oring = ctx.enter_context(tc.tile_pool(name="oring", bufs=8))
        sm = ctx.enter_context(tc.tile_pool(name="sm", bufs=1))

        # constants: f32r identity + broadcast gamma + bf16 warmup scratch
        ones = const.tile([P, P], f32)
        nc.vector.memset(ones[:], 1.0)
        ident = const.tile([P, P], f32)
        nc.gpsimd.affine_select(
            ident[:], ones[:], pattern=[[1, P]],
            compare_op=mybir.AluOpType.is_equal, fill=0.0,
            base=0, channel_multiplier=-1,
        )
        identr = const.tile([P, P], f32r)
        nc.vector.tensor_copy(identr[:], ident[:])
        warm = const.tile([P, C], bf16)
        nc.vector.memset(warm[:], 0.5)

        g_sb = const.tile([1, 1], f32)
        nc.sync.dma_start(g_sb[0:1, 0:1], g_in[0:1])
        g_bc = const.tile([P, 1], f32)
        nc.gpsimd.partition_broadcast(g_bc[:], g_sb[0:1, :])

        a_all = abig.tile([P, NT * C], f32r)

        with tc.tile_pool(name="psD", bufs=1, space="PSUM") as psD:
            # HAM warmup: keep PE busy with dummy bf16 matmuls while the
            # first DMAs land (~3.5us to flip the clock gate to 2.4 GHz).
            wps = psD.tile([P, C], f32)
            for _ in range(18):
                nc.tensor.matmul(wps[:], warm[:, 0:P], warm[:],
                                 start=True, stop=True)

            with tc.tile_pool(name="psA", bufs=2, space="PSUM") as psA:
                aTa_ps = [psA.tile([P, C], f32, tag="aTa", name=f"aTa{k}")
                          for k in range(2)]

                # ---- Phase A: load a + accumulate aTa ----
                for d in range(ND):
                    a_dt = a_all[:, d * 2 * C:(d + 1) * 2 * C]
                    nc.sync.dma_start(
                        a_dt.rearrange("p (j c) -> p j c", j=2),
                        x_in[ts(d, 2 * P), :].bitcast(f32r).rearrange(
                            "(j p) c -> p j c", p=P
                        ),
                    )
                    for j in range(2):
                        i = 2 * d + j
                        a_i = a_dt[:, j * C:(j + 1) * C]
                        for k in range(2):
                            nc.tensor.matmul(
                                aTa_ps[k][:],
                                a_i[:, ts(k, P)],
                                a_i[:],
                                start=(i == 0),
                                stop=(i == NT - 1),
                                skip_group_check=True,
                            )

                # ---- Softmax -> M = gamma * attn + I ----
                Ms = []
                for k in range(2):
                    negmx = sm.tile([P, 1], f32, name=f"negmx{k}")
                    nc.vector.tensor_reduce(
                        out=negmx[:], in_=aTa_ps[k][:], op=mybir.AluOpType.max,
                        axis=mybir.AxisListType.X, negate=True,
                    )
                    e = sm.tile([P, C], f32, name=f"e{k}")
                    s = sm.tile([P, 1], f32, name=f"s{k}")
                    nc.scalar.activation(
                        e[:], aTa_ps[k][:], mybir.ActivationFunctionType.Exp,
                        bias=negmx[:, 0:1], scale=1.0, accum_out=s[:],
                    )
                    r = sm.tile([P, 1], f32, name=f"r{k}")
                    nc.vector.reciprocal(r[:], s[:])
                    rg = sm.tile([P, 1], f32, name=f"rg{k}")
                    nc.vector.tensor_mul(rg[:], r[:], g_bc[:])
                    Mk = sm.tile([P, C], f32r, name=f"M{k}")
                    nc.vector.tensor_scalar_mul(Mk[:], e[:], rg[:, 0:1])
                    nc.vector.tensor_add(Mk[:, ts(k, P)], Mk[:, ts(k, P)],
                                         identr[:])
                    Ms.append(Mk)

        with (
            tc.tile_pool(name="psT", bufs=3, space="PSUM") as psT,
            tc.tile_pool(name="psO", bufs=5, space="PSUM") as psO,
        ):
            # ---- Phase B: transpose + out = a @ M ----
            for d in range(ND):
                a_dt = a_all[:, d * 2 * C:(d + 1) * 2 * C]
                tp = psT.tile([P, 2 * C], f32r, name=f"tp{d}", tag="tp")
                for j in range(2):
                    for k in range(2):
                        nc.tensor.transpose(
                            tp[:, j * C + k * P: j * C + (k + 1) * P],
                            a_dt[:, j * C + k * P: j * C + (k + 1) * P],
                            identr[:],
                        )
                aTr = tring.tile([P, 2 * C], f32r, name=f"aTr{d}", tag="aTr")
                nc.vector.tensor_copy(aTr[:, 0:C], tp[:, 0:C])
                nc.scalar.copy(aTr[:, C:2 * C], tp[:, C:2 * C])

                o_dt = oring.tile([P, 2 * C], f32, name=f"o{d}", tag="o")
                ops = psO.tile([P, 2 * C], f32, name=f"ops{d}", tag="ops")
                for j in range(2):
                    for k in range(2):
                        nc.tensor.matmul(
                            ops[:, ts(j, C)],
                            aTr[:, j * C + k * P: j * C + (k + 1) * P],
                            Ms[k][:],
                            start=(k == 0),
                            stop=(k == 1),
                        )
                nc.vector.tensor_copy(o_dt[:, 0:C], ops[:, 0:C])
                nc.scalar.copy(o_dt[:, C:2 * C], ops[:, C:2 * C])
                nc.sync.dma_start(
                    y_out[ts(d, 2 * P), :].rearrange("(j p) c -> p j c", p=P),
                    o_dt[:].rearrange("p (j c) -> p j c", j=2),
                )


_CACHE = {}


def _build():
    nc = bacc.Bacc("TRN2", target_bir_lowering=False, debug=False,
                   num_devices=N_CORES)
    x_in = nc.dram_tensor("x", [HW, C], f32, kind="ExternalInput").ap()
    g_in = nc.dram_tensor("gamma", [1], f32, kind="ExternalInput").ap()
    y_out = nc.dram_tensor("y", [HW, C], f32, kind="ExternalOutput").ap()
    with tile.TileContext(nc) as tc:
        _cam_body(tc, y_out, x_in, g_in)
    nc.compile()
    return nc


def _run(x, gamma, trace=False):
    if "nc" not in _CACHE:
        _CACHE["nc"] = _build()
    nc = _CACHE["nc"]
    xs = np.ascontiguousarray(np.asarray(x, dtype=np.float32).reshape(B, HW, C))
    g = np.ascontiguousarray(np.asarray(gamma, dtype=np.float32).reshape(1))
    in_maps = [{"x": xs[b], "gamma": g} for b in range(B)]
    return run_bass_kernel_spmd(nc, in_maps, core_ids=list(range(N_CORES)),
                                trace=trace)


def kernel(x, gamma):
    res = _run(x, gamma, trace=False)
    out = np.stack([res.results[b]["y"] for b in range(B)], axis=0)
    return out.reshape(B, H, W, C).astype(np.float32)


# revision 14
# speedup vs baseline: 1.0386x; 1.0386x over previous
"""CAM (channel attention) kernel for Trainium2, data-parallel over batch.

out[b] = gamma * (a[b] @ softmax(a[b]^T a[b])) + x[b],  a[b] = x[b].reshape(HW, C)

Per core (one batch element):
  Phase A: stream a into a resident SBUF buffer in [128, 512] double-tiles
           and accumulate aTa = a^T a in PSUM (f32r matmuls, K=16384 over
           128 accumulation steps). DMA-bound.
  Softmax: row-softmax of aTa folded into M = gamma * attn + I, so
           out = a @ M (residual + gamma fused into the small matrix).
  Phase B: per double-tile, PE-transpose a-chunks (f32r transpose-mode,
           value-preserving) into a packed PSUM bank, evacuate to a small
           aT ring (DVE+ACT halves), then out rows = aT_chunk.T @ M
           (f32r matmuls, K=256), evacuate PSUM -> SBUF -> DRAM.
           The transposes only depend on a, so the PE works through the
           softmax barrier without going cold.
Dummy bf16 matmuls warm the PE clock gate (HAM) at kernel start.
"""

import sys

import numpy as np

for _p in ("/opt/trn_rl_repo",):
    if _p not in sys.path:
        sys.path.insert(0, _p)

import concourse.bass as bass
import concourse.tile as tile
from concourse import bacc, mybir
from concourse.bass_utils import run_bass_kernel_spmd

B, H, W, C = 8, 128, 128, 256
HW = H * W
P = 128
NT = HW // P          # 128 row-tiles of a
ND = NT // 2          # 64 double-tiles
N_CORES = 8

f32 = mybir.dt.float32
f32r = mybir.dt.float32r
bf16 = mybir.dt.bfloat16
ts = bass.ts


def _cam_body(tc, y_out, x_in, g_in):
    nc = tc.nc
    import contextlib

    with contextlib.ExitStack() as ctx:
        const = ctx.enter_context(tc.tile_pool(name="const", bufs=1))
        abig = ctx.enter_context(tc.tile_pool(name="abig", bufs=1))
        tring = ctx.enter_context(tc.tile_pool(name="tring", bufs=4))
        oring = ctx.enter_context(tc.tile_pool(name="oring", bufs=8))
        sm = ctx.enter_context(tc.tile_pool(name="sm", bufs=1))

        # constants: f32r identity + broadcast gamma + bf16 warmup scratch
        ones = const.tile([P, P], f32)
        nc.vector.memset(ones[:], 1.0)
        ident = const.tile([P, P], f32)
        nc.gpsimd.affine_select(
            ident[:], ones[:], pattern=[[1, P]],
            compare_op=mybir.AluOpType.is_equal, fill=0.0,
            base=0, channel_multiplier=-1,
        )
        identr = const.tile([P, P], f32r)
        nc.vector.tensor_copy(identr[:], ident[:])
        warm = const.tile([P, C], bf16)
        nc.vector.memset(warm[:], 0.5)

        g_sb = const.tile([1, 1], f32)
        nc.sync.dma_start(g_sb[0:1, 0:1], g_in[0:1])
        g_bc = const.tile([P, 1], f32)
        nc.gpsimd.partition_broadcast(g_bc[:], g_sb[0:1, :])

        a_all = abig.tile([P, NT * C], f32r)
        SPLIT = 14
        aT_first = abig.tile([P, SPLIT * 2 * C], f32r)

        def transposes_for(d, tpool):
            a_dt = a_all[:, d * 2 * C:(d + 1) * 2 * C]
            tp = tpool.tile([P, 2 * C], f32r, name=f"tp{d}", tag="tp")
            for j in range(2):
                for k in range(2):
                    nc.tensor.transpose(
                        tp[:, j * C + k * P: j * C + (k + 1) * P],
                        a_dt[:, j * C + k * P: j * C + (k + 1) * P],
                        identr[:],
                    )
            if d < SPLIT:
                aTr = aT_first[:, d * 2 * C:(d + 1) * 2 * C]
            else:
                aTr = tring.tile([P, 2 * C], f32r, name=f"aTr{d}", tag="aTr")
            nc.vector.tensor_copy(aTr[:, 0:C], tp[:, 0:C])
            nc.scalar.copy(aTr[:, C:2 * C], tp[:, C:2 * C])
            return aTr

        psT = ctx.enter_context(tc.tile_pool(name="psT", bufs=3, space="PSUM"))
        with tc.tile_pool(name="psD", bufs=1, space="PSUM") as psD:
            # HAM warmup: keep PE busy with dummy bf16 matmuls while the
            # first DMAs land (~3.5us to flip the clock gate to 2.4 GHz).
            wps = psD.tile([P, C], f32)
            for _ in range(8):
                nc.tensor.matmul(wps[:], warm[:, 0:P], warm[:],
                                 start=True, stop=True)

            aTr_tiles = {}
            with tc.tile_pool(name="psA", bufs=2, space="PSUM") as psA:
                aTa_ps = [psA.tile([P, C], f32, tag="aTa", name=f"aTa{k}")
                          for k in range(2)]

                # ---- Phase A: load a + accumulate aTa ----
                for d in range(ND):
                    a_dt = a_all[:, d * 2 * C:(d + 1) * 2 * C]
                    nc.sync.dma_start(
                        a_dt.rearrange("p (j c) -> p j c", j=2),
                        x_in[ts(d, 2 * P), :].bitcast(f32r).rearrange(
                            "(j p) c -> p j c", p=P
                        ),
                    )
                    for j in range(2):
                        i = 2 * d + j
                        a_i = a_dt[:, j * C:(j + 1) * C]
                        for k in range(2):
                            nc.tensor.matmul(
                                aTa_ps[k][:],
                                a_i[:, ts(k, P)],
                                a_i[:],
                                start=(i == 0),
                                stop=(i == NT - 1),
                                skip_group_check=True,
                            )
                    if d < SPLIT:
                        aTr_tiles[d] = transposes_for(d, psT)

                # ---- Softmax -> M = gamma * attn + I ----
                Ms = []
                for k in range(2):
                    negmx = sm.tile([P, 1], f32, name=f"negmx{k}")
                    nc.vector.tensor_reduce(
                        out=negmx[:], in_=aTa_ps[k][:], op=mybir.AluOpType.max,
                        axis=mybir.AxisListType.X, negate=True,
                    )
                    e = sm.tile([P, C], f32, name=f"e{k}")
                    s = sm.tile([P, 1], f32, name=f"s{k}")
                    nc.scalar.activation(
                        e[:], aTa_ps[k][:], mybir.ActivationFunctionType.Exp,
                        bias=negmx[:, 0:1], scale=1.0, accum_out=s[:],
                    )
                    r = sm.tile([P, 1], f32, name=f"r{k}")
                    nc.vector.reciprocal(r[:], s[:])
                    rg = sm.tile([P, 1], f32, name=f"rg{k}")
                    nc.vector.tensor_mul(rg[:], r[:], g_bc[:])
                    Mk = sm.tile([P, C], f32r, name=f"M{k}")
                    nc.vector.tensor_scalar_mul(Mk[:], e[:], rg[:, 0:1])
                    nc.vector.tensor_add(Mk[:, ts(k, P)], Mk[:, ts(k, P)],
                                         identr[:])
                    Ms.append(Mk)

        with tc.tile_pool(name="psO", bufs=5, space="PSUM") as psO:
            # ---- Phase B: remaining transposes + out = a @ M ----
            for d in range(ND):
                aTr = (aT_first[:, d * 2 * C:(d + 1) * 2 * C]
                       if d < SPLIT else transposes_for(d, psT))

                o_dt = oring.tile([P, 2 * C], f32, name=f"o{d}", tag="o")
                ops = psO.tile([P, 2 * C], f32, name=f"ops{d}", tag="ops")
                for j in range(2):
                    for k in range(2):
                        nc.tensor.matmul(
                            ops[:, ts(j, C)],
                            aTr[:, j * C + k * P: j * C + (k + 1) * P],
                            Ms[k][:],
                            start=(k == 0),
                            stop=(k == 1),
                        )
                nc.vector.tensor_copy(o_dt[:, 0:C], ops[:, 0:C])
                nc.scalar.copy(o_dt[:, C:2 * C], ops[:, C:2 * C])
                nc.sync.dma_start(
                    y_out[ts(d, 2 * P), :].rearrange("(j p) c -> p j c", p=P),
                    o_dt[:].rearrange("p (j c) -> p j c", j=2),
                )


_CACHE = {}


def _build():
    nc = bacc.Bacc("TRN2", target_bir_lowering=False, debug=False,
                   num_devices=N_CORES)
    x_in = nc.dram_tensor("x", [HW, C], f32, kind="ExternalInput").ap()
    g_in = nc.dram_tensor("gamma", [1], f32, kind="ExternalInput").ap()
    y_out = nc.dram_tensor("y", [HW, C], f32, kind="ExternalOutput").ap()
    with tile.TileContext(nc) as tc:
        _cam_body(tc, y_out, x_in, g_in)
    nc.compile()
    return nc


def _run(x, gamma, trace=False):
    if "nc" not in _CACHE:
        _CACHE["nc"] = _build()
    nc = _CACHE["nc"]
    xs = np.ascontiguousarray(np.asarray(x, dtype=np.float32).reshape(B, HW, C))
    g = np.ascontiguousarray(np.asarray(gamma, dtype=np.float32).reshape(1))
    in_maps = [{"x": xs[b], "gamma": g} for b in range(B)]
    return run_bass_kernel_spmd(nc, in_maps, core_ids=list(range(N_CORES)),
                                trace=trace)


def kernel(x, gamma):
    res = _run(x, gamma, trace=False)
    out = np.stack([res.results[b]["y"] for b in range(B)], axis=0)
    return out.reshape(B, H, W, C).astype(np.float32)


# revision 19
# speedup vs baseline: 1.1247x; 1.0829x over previous
"""CAM (channel attention) kernel for Trainium2, data-parallel over batch.

out[b] = gamma * (a[b] @ softmax(a[b]^T a[b])) + x[b],  a[b] = x[b].reshape(HW, C)

Per core (one batch element):
  Layout: rows are distributed 4-consecutive-per-partition (row 4t+r on
  partition t, free block r), so every DMA touches DRAM strictly
  sequentially (4 KB runs per partition visit). The row permutation is
  irrelevant to aTa (it sums over all rows) and is applied symmetrically
  on input and output.

  Phase A: stream a into a resident SBUF buffer in 512-row chunks and
           accumulate aTa = a^T a in PSUM (f32r matmuls, K=16384 over 128
           row-groups).  Also PE-transpose the first SPLITQ chunks.
  Softmax: row-softmax of aTa folded into M = gamma * attn + I, so
           out = a @ M (residual + gamma fused into the small matrix).
  Phase B: remaining transposes + out rows = aT_group.T @ M (f32r
           matmuls, K=256), evacuated PSUM -> SBUF -> DRAM in 512-row
           chunks.  Transposes are done in place in the resident buffer
           (each slice is dead once its aTa matmuls have read it), so the
           phase split needs no extra SBUF.
Dummy bf16 matmuls warm the PE clock gate (HAM) at kernel start; the
phase-B transposes keep it warm across the softmax barrier.
"""

import sys

import numpy as np

for _p in ("/opt/trn_rl_repo",):
    if _p not in sys.path:
        sys.path.insert(0, _p)

import concourse.bass as bass
import concourse.tile as tile
from concourse import bacc, mybir
from concourse.bass_utils import run_bass_kernel_spmd

B, H, W, C = 8, 128, 128, 256
HW = H * W
P = 128
NQ = HW // (4 * P)    # 32 chunks of 512 rows
N_CORES = 8
SPLITQ = 17           # chunks whose transposes happen in phase A

f32 = mybir.dt.float32
f32r = mybir.dt.float32r
bf16 = mybir.dt.bfloat16
ts = bass.ts


def _cam_body(tc, y_out, x_in, g_in):
    nc = tc.nc
    import contextlib

    with contextlib.ExitStack() as ctx:
        const = ctx.enter_context(tc.tile_pool(name="const", bufs=1))
        abig = ctx.enter_context(tc.tile_pool(name="abig", bufs=1))
        oring = ctx.enter_context(tc.tile_pool(name="oring", bufs=4))
        sm = ctx.enter_context(tc.tile_pool(name="sm", bufs=1))

        # constants: f32r identity + broadcast gamma + bf16 warmup scratch
        ones = const.tile([P, P], f32)
        nc.vector.memset(ones[:], 1.0)
        ident = const.tile([P, P], f32)
        nc.gpsimd.affine_select(
            ident[:], ones[:], pattern=[[1, P]],
            compare_op=mybir.AluOpType.is_equal, fill=0.0,
            base=0, channel_multiplier=-1,
        )
        identr = const.tile([P, P], f32r)
        nc.vector.tensor_copy(identr[:], ident[:])
        warm = const.tile([P, C], bf16)
        nc.vector.memset(warm[:], 0.5)

        g_sb = const.tile([1, 1], f32)
        nc.sync.dma_start(g_sb[0:1, 0:1], g_in[0:1])
        g_bc = const.tile([P, 1], f32)
        nc.gpsimd.partition_broadcast(g_bc[:], g_sb[0:1, :])

        # resident a buffer: chunk q at columns [q*4C, (q+1)*4C), group g of
        # rows {4t+g} at sub-columns [g*C, (g+1)*C)
        a_all = abig.tile([P, NQ * 4 * C], f32r)

        def transposes_for(q, h, tpool):
            """Transpose group pair h (groups 2h, 2h+1) of chunk q in place."""
            a_gp = a_all[:, (q * 4 + 2 * h) * C:(q * 4 + 2 * h + 2) * C]
            tp = tpool.tile([P, 2 * C], f32r, name=f"tp{q}_{h}", tag="tp")
            for g in range(2):
                for k in range(2):
                    nc.tensor.transpose(
                        tp[:, g * C + k * P: g * C + (k + 1) * P],
                        a_gp[:, g * C + k * P: g * C + (k + 1) * P],
                        identr[:],
                    )
            nc.vector.tensor_copy(a_gp[:, 0:C], tp[:, 0:C])
            nc.scalar.copy(a_gp[:, C:2 * C], tp[:, C:2 * C])

        with tc.tile_pool(name="psD", bufs=1, space="PSUM") as psD:
            # HAM warmup: keep PE busy with dummy bf16 matmuls while the
            # first DMAs land (~3us to flip the clock gate to 2.4 GHz).
            wps = psD.tile([P, C], f32)
            for _ in range(8):
                nc.tensor.matmul(wps[:], warm[:, 0:P], warm[:],
                                 start=True, stop=True)

            with (
                tc.tile_pool(name="psA", bufs=2, space="PSUM") as psA,
                tc.tile_pool(name="psTa", bufs=3, space="PSUM") as psTa,
            ):
                aTa_ps = [psA.tile([P, C], f32, tag="aTa", name=f"aTa{k}")
                          for k in range(2)]

                # ---- Phase A: load a + accumulate aTa ----
                for q in range(NQ):
                    a_qt = a_all[:, q * 4 * C:(q + 1) * 4 * C]
                    nc.sync.dma_start(
                        a_qt.rearrange("t (r c) -> t r c", r=4),
                        x_in[ts(q, 4 * P), :].bitcast(f32r).rearrange(
                            "(t r) c -> t r c", r=4
                        ),
                    )
                    for g in range(4):
                        i = 4 * q + g
                        a_i = a_qt[:, g * C:(g + 1) * C]
                        for k in range(2):
                            nc.tensor.matmul(
                                aTa_ps[k][:],
                                a_i[:, ts(k, P)],
                                a_i[:],
                                start=(i == 0),
                                stop=(i == 4 * NQ - 1),
                                skip_group_check=True,
                            )
                    if q < SPLITQ:
                        for h in range(2):
                            transposes_for(q, h, psTa)

                # ---- Softmax -> M = gamma * attn + I ----
                Ms = []
                for k in range(2):
                    negmx = sm.tile([P, 1], f32, name=f"negmx{k}")
                    nc.vector.tensor_reduce(
                        out=negmx[:], in_=aTa_ps[k][:], op=mybir.AluOpType.max,
                        axis=mybir.AxisListType.X, negate=True,
                    )
                    e = sm.tile([P, C], f32, name=f"e{k}")
                    s = sm.tile([P, 1], f32, name=f"s{k}")
                    nc.scalar.activation(
                        e[:], aTa_ps[k][:], mybir.ActivationFunctionType.Exp,
                        bias=negmx[:, 0:1], scale=1.0, accum_out=s[:],
                    )
                    r = sm.tile([P, 1], f32, name=f"r{k}")
                    nc.vector.reciprocal(r[:], s[:])
                    rg = sm.tile([P, 1], f32, name=f"rg{k}")
                    nc.vector.tensor_mul(rg[:], r[:], g_bc[:])
                    Mk = sm.tile([P, C], f32r, name=f"M{k}")
                    nc.vector.tensor_scalar_mul(Mk[:], e[:], rg[:, 0:1])
                    nc.vector.tensor_add(Mk[:, ts(k, P)], Mk[:, ts(k, P)],
                                         identr[:])
                    Ms.append(Mk)

        with (
            tc.tile_pool(name="psT", bufs=3, space="PSUM") as psT,
            tc.tile_pool(name="psO", bufs=4, space="PSUM") as psO,
        ):
            # ---- Phase B: remaining transposes + out = a @ M ----
            for q in range(NQ):
                o_qt = oring.tile([P, 4 * C], f32, name=f"o{q}", tag="o")
                for h in range(2):
                    if q >= SPLITQ:
                        transposes_for(q, h, psT)
                    ops = psO.tile([P, 2 * C], f32, name=f"ops{q}_{h}",
                                   tag="ops")
                    for g in range(2):
                        i0 = (q * 4 + 2 * h + g) * C
                        for k in range(2):
                            nc.tensor.matmul(
                                ops[:, ts(g, C)],
                                a_all[:, i0 + k * P: i0 + (k + 1) * P],
                                Ms[k][:],
                                start=(k == 0),
                                stop=(k == 1),
                            )
                    o_h = o_qt[:, 2 * h * C:(2 * h + 2) * C]
                    nc.vector.tensor_copy(o_h[:, 0:C], ops[:, 0:C])
                    nc.scalar.copy(o_h[:, C:2 * C], ops[:, C:2 * C])
                nc.sync.dma_start(
                    y_out[ts(q, 4 * P), :].rearrange("(t r) c -> t r c", r=4),
                    o_qt[:].rearrange("t (r c) -> t r c", r=4),
                )


_CACHE = {}


def _build():
    nc = bacc.Bacc("TRN2", target_bir_lowering=False, debug=False,
                   enable_asserts=False, num_devices=N_CORES)
    x_in = nc.dram_tensor("x", [HW, C], f32, kind="ExternalInput").ap()
    g_in = nc.dram_tensor("gamma", [1], f32, kind="ExternalInput").ap()
    y_out = nc.dram_tensor("y", [HW, C], f32, kind="ExternalOutput").ap()
    with tile.TileContext(nc) as tc:
        _cam_body(tc, y_out, x_in, g_in)
    nc.compile()
    return nc


def _run(x, gamma, trace=False):
    if "nc" not in _CACHE:
        _CACHE["nc"] = _build()
    nc = _CACHE["nc"]
    xs = np.ascontiguousarray(np.asarray(x, dtype=np.float32).reshape(B, HW, C))
    g = np.ascontiguousarray(np.asarray(gamma, dtype=np.float32).reshape(1))
    in_maps = [{"x": xs[b], "gamma": g} for b in range(B)]
    return run_bass_kernel_spmd(nc, in_maps, core_ids=list(range(N_CORES)),
                                trace=trace)


def kernel(x, gamma):
    res = _run(x, gamma, trace=False)
    out = np.stack([res.results[b]["y"] for b in range(B)], axis=0)
    return out.reshape(B, H, W, C).astype(np.float32)
